# revision 10
# baseline (speedup 1.0000x reference)
"""Trainium2 Bass kernel for the nn_Adaptor problem.

Computation (per batch image):
  avgpool4x4 -> GN(32 groups)+SiLU -> conv3x3 320->8 -> attention(4 heads) ->
  per-pixel LN + MLP -> GN(8)+SiLU -> upsample x4 nearest -> conv3x3 8->320

Distribution: pure data parallel over batch. 16 images / 8 cores = 2 per core.
Params are baked into the NEFF as inline consts (recomputed from the numpy
arrays passed to kernel() at trace time).

Key implementation tricks:
  - pooling keeps raw 4x4 sums (16x scale); GN1 uses eps_eff = 256*eps so the
    normalized output is exact.
  - GN1 group stats via per-channel bn_stats + grouping-matrix matmuls on PE
    (avoids cross-partition reductions on DVE).
  - GN1/GN2 + SiLU fused into a single scalar-engine activation per tile
    (out = Silu(x*scale_c + bias_c)).
  - conv1 as 9 shifted-window matmuls over a zero-padded 18x18 tile.
  - attention computes transposed scores E^T = exp(k^T q) directly (scores are
    |s| < 0.5 for this operator family, so no max subtraction), row sums via
    ones-vector matmuls, attn@V via a PE transpose of V, normalization at the
    end.
  - GN2 is applied pre-upsample (nearest-upsample preserves per-channel
    mean/var), so the whole middle runs on 16x16 grids.
  - conv2-after-upsample collapses to 9 phase groups with summed weights on the
    16x16 grid; each group is one K<=40 matmul per 128-channel output chunk.
    Phase outputs are interleaved on-chip (strided DVE/ACT copies with
    broadcast duplication), then written out with fully contiguous DMAs.
"""

import numpy as np

import concourse.bass as bass
import concourse.bacc as bacc
import concourse.tile as tile
from concourse import mybir
from concourse.bass_utils import run_bass_kernel_spmd

F32 = mybir.dt.float32
AF = mybir.ActivationFunctionType
ALU = mybir.AluOpType
AX = mybir.AxisListType

CH, C, D, HEADS = 320, 4, 8, 4
EPS = 1e-5
B = 2          # local batch per core
N = 256        # 16*16 tokens
NCORES = 8

# conv2 phase classes: pi -> list of (padded-window row offset, kernel rows summed)
TAPSETS = {
    0: [(0, (0,)), (1, (1, 2))],
    1: [(1, (0, 1, 2))],
    3: [(1, (0, 1)), (2, (2,))],
}
GROUPS = [(pi, pj) for pi in (0, 1, 3) for pj in (0, 1, 3)]
ROWSETS = {0: (0, 1), 1: (1, 2), 3: (3, 1)}   # (start, count) of duplicated rows

KERNEL_TRACE = False


def _conv2_tables(w_conv2):
    """Collapsed per-phase-group weights.

    Returns W9 [72, 9, 3, 128] f32: contraction rows = (slot, ic) over the full
    9-slot shifted-window stack (unused slots zero), col = oc within chunk.
    Chunk 2 (64 real oc) is duplicated into cols 64:128 so the combined
    two-batch chunk-2 matmul can target psum partitions 64:128.
    """
    W9 = np.zeros((72, 9, 3, 128), np.float32)
    for g, (pi, pj) in enumerate(GROUPS):
        ty, tx = TAPSETS[pi], TAPSETS[pj]
        for (dy, kys) in ty:
            for (dx, kxs) in tx:
                s = 3 * dy + dx
                wsum = np.zeros((CH, D), np.float32)
                for ky in kys:
                    for kx in kxs:
                        wsum += w_conv2[:, :, ky, kx]
                for c in range(3):
                    oc0 = 128 * c
                    ocn = min(128, CH - oc0)
                    blk = wsum[oc0:oc0 + ocn].T  # [8, ocn]
                    W9[8 * s:8 * s + 8, g, c, :ocn] += blk
                    if c == 2:
                        W9[8 * s:8 * s + 8, g, c, 64:64 + ocn] += blk
    return W9


def build(params):
    P = params
    nc = bacc.Bacc("TRN2")

    x = nc.dram_tensor("x", [B, CH, 64, 64], F32, kind="ExternalInput")
    out = nc.dram_tensor("out", [B, CH, 64, 64], F32, kind="ExternalOutput")

    # ---------------- host-side constant prep ----------------
    s = float(1 / np.sqrt(D // HEADS))
    wq = P["w_qkv"].copy()
    bq = P["b_qkv"].copy()
    wq[D:2 * D] *= s
    bq[D:2 * D] *= s

    # conv1 weights: [128, 3, 9, 8]; chunk-2 tail rows zero
    W1 = np.zeros((128, 3, 9, 8), np.float32)
    for c in range(3):
        c0 = 128 * c
        cn = min(128, CH - c0)
        for ky in range(3):
            for kx in range(3):
                W1[:cn, c, 3 * ky + kx, :] = P["w_conv1"][:, c0:c0 + cn, ky, kx].T

    W9 = _conv2_tables(P["w_conv2"])

    # grouping matrices for GN1 (32 groups of 10 channels)
    Gsum = np.zeros((128, 3, 32), np.float32)
    GT = np.zeros((32, 3, 128), np.float32)
    for c in range(CH):
        k, p = divmod(c, 128)
        Gsum[p, k, c // 10] = 0.1
        GT[c // 10, k, p] = 1.0

    # per-channel vectors in [128, 3] chunk layout
    def chunks(v, dup2=False):
        a = np.zeros((128, 3), np.float32)
        for c in range(3):
            c0 = 128 * c
            cn = min(128, CH - c0)
            a[:cn, c] = v[c0:c0 + cn]
            if dup2 and c == 2:
                a[64:64 + cn, c] = v[c0:c0 + cn]
        return a

    g1c = chunks(P["g1"])
    b1c = chunks(P["b1"])
    b2c = chunks(P["b_conv2"], dup2=True)

    # packed small consts on 8 partitions: [8, X]
    cols8 = {}
    pk8 = []

    def pack8(name, arr):
        arr = np.asarray(arr, np.float32).reshape(8, -1)
        c0 = sum(a.shape[1] for a in pk8)
        cols8[name] = (c0, arr.shape[1])
        pk8.append(arr)

    # q/k projections scattered so head h lands at partition base 32h
    # (PE matmul quadrant rule: sub-32 operands must sit at base 0/32/64/96)
    WqS = np.zeros((8, 128), np.float32)
    WkS = np.zeros((8, 128), np.float32)
    bqS = np.zeros((128, 1), np.float32)
    bkS = np.zeros((128, 1), np.float32)
    WpS = np.zeros((128, 8), np.float32)
    for h in range(4):
        for cc in range(2):
            WqS[:, 32 * h + cc] = wq[2 * h + cc, :]
            WkS[:, 32 * h + cc] = wq[D + 2 * h + cc, :]
            bqS[32 * h + cc, 0] = bq[2 * h + cc]
            bkS[32 * h + cc, 0] = bq[D + 2 * h + cc]
            WpS[32 * h + cc, :] = P["w_proj"][:, 2 * h + cc]

    pack8("wqS", WqS)
    pack8("wkS", WkS)
    pack8("wvT", P["w_qkv"][2 * D:3 * D].T)   # [8, 8]
    pack8("ident8", np.eye(8, dtype=np.float32))
    pack8("wf1T", P["w_fc1"].T)        # [8, 16]
    pack8("ones8", np.ones((8, 1), np.float32))
    pack8("bv", bq[2 * D:3 * D].reshape(8, 1))
    for nm in ("b_conv1", "b_proj", "ln_g", "ln_b", "b_fc2", "ga", "ba", "g2", "b2"):
        pack8(nm, P[nm].reshape(8, 1))
    PK8 = np.concatenate(pk8, axis=1)

    PK16 = np.concatenate([P["w_fc2"].T, P["b_fc1"].reshape(16, 1)], axis=1)  # [16, 9]

    # [128, X] pack: bqS, bkS, ones x2, WpS
    PK128 = np.concatenate(
        [bqS, bkS, np.ones((128, 2), np.float32), WpS], axis=1)  # [128, 12]

    h_w1 = nc.inline_tensor(W1, "cW1")
    h_w9 = nc.inline_tensor(W9, "cW9")
    h_gsum = nc.inline_tensor(Gsum, "cGsum")
    h_gt = nc.inline_tensor(GT, "cGT")
    h_vchunk = nc.inline_tensor(np.stack([g1c, b1c, b2c], axis=2), "cVch")  # [128,3,3]
    h_pk8 = nc.inline_tensor(PK8, "cPK8")
    h_pk16 = nc.inline_tensor(PK16, "cPK16")
    h_pk128 = nc.inline_tensor(PK128, "cPK128")
    h_one18 = nc.inline_tensor(np.ones((1, 8), np.float32), "cOne18")

    with tile.TileContext(nc) as tc:
        with (
            tc.tile_pool(name="consts", bufs=1) as csts,
            tc.tile_pool(name="xin", bufs=3) as xin,
            tc.tile_pool(name="pooltmp", bufs=2) as ptmp,
            tc.tile_pool(name="mid", bufs=1) as mid,
            tc.tile_pool(name="midb", bufs=2) as midb,
            tc.tile_pool(name="et", bufs=4) as etp,
            tc.tile_pool(name="outp", bufs=3) as outp,
            # PSUM budget is 8 banks; every tile rounds up to a bank.
            # psA: all small sequential psums (one shared tag, 2 slots)
            # psT: transposes + transposed-score tiles (one shared tag, 2 slots)
            # psAcc: softmax-denominator + EV accumulators (2 tags x 1 slot)
            # psC: conv2 output tiles (2 slots)
            tc.tile_pool(name="psA", bufs=2, space="PSUM") as psA,
            tc.tile_pool(name="psT", bufs=2, space="PSUM") as psT,
            tc.tile_pool(name="psAcc", bufs=1, space="PSUM") as psAcc,
            tc.tile_pool(name="psC", bufs=2, space="PSUM") as psC,
        ):
            # ---------------- consts to SBUF ----------------
            w1t = csts.tile([128, 3, 9, 8], F32)
            nc.sync.dma_start(out=w1t, in_=h_w1[:])
            w9t = csts.tile([72, 9, 3, 128], F32)
            nc.sync.dma_start(out=w9t, in_=h_w9[:])
            gsumt = csts.tile([128, 3, 32], F32)
            nc.sync.dma_start(out=gsumt, in_=h_gsum[:])
            gtt = csts.tile([32, 3, 128], F32)
            nc.sync.dma_start(out=gtt, in_=h_gt[:])
            vcht = csts.tile([128, 3, 3], F32)
            nc.sync.dma_start(out=vcht, in_=h_vchunk[:])
            pk8t = csts.tile([8, PK8.shape[1]], F32)
            nc.sync.dma_start(out=pk8t, in_=h_pk8[:])
            pk16t = csts.tile([16, 9], F32)
            nc.sync.dma_start(out=pk16t, in_=h_pk16[:])
            pk128t = csts.tile([128, 12], F32)
            nc.sync.dma_start(out=pk128t, in_=h_pk128[:])
            one18t = csts.tile([1, 8], F32)
            nc.sync.dma_start(out=one18t, in_=h_one18[:])
            bqS_t = pk128t[:, 0:1]
            bkS_t = pk128t[:, 1:2]
            ones2t = pk128t[:, 2:4]
            wpS_t = pk128t[:, 4:12]

            def c8(name):
                c0, w = cols8[name]
                return pk8t[:, c0:c0 + w]

            wf2T = pk16t[:, 0:8]
            bf1 = pk16t[:, 8:9]

            eps1 = csts.tile([32, 1], F32)
            nc.vector.memset(eps1, 256.0 * EPS)
            eps8 = csts.tile([8, 1], F32)
            nc.vector.memset(eps8, EPS)
            eps1p = csts.tile([1, 1], F32)
            nc.vector.memset(eps1p, EPS)

            # ---------------- per-core state tiles ----------------
            z0 = mid.tile([128, 3, B, N], F32)     # pooled sums (16x scale)
            nc.vector.memset(z0[64:128, 2, :, :], 0.0)
            stat2 = mid.tile([128, 3, B, 2], F32)  # per-channel (mean, E[x^2])
            ab = mid.tile([32, 2 * B], F32)        # per-group (rstd', mean') per batch
            sbias = mid.tile([128, 3, B, 2], F32)  # per-channel GN1 (scale, bias)
            zc = mid.tile([8, B, N], F32)          # conv1 out
            qs4 = mid.tile([128, B, N], F32)       # q, head h at partitions 32h:32h+2
            ks4 = mid.tile([128, B, N], F32)       # k (pre-scaled), same layout
            vsb = mid.tile([8, B, N], F32)
            vT = mid.tile([128, 2, B, 8], F32)     # v^T per 128-row m-chunk
            z1 = mid.tile([8, B, N], F32)          # post-attention residual
            z2 = mid.tile([8, B, N], F32)          # post-MLP residual

            # ---------------- phase 1: load + pool ----------------
            # order: b0 chunks 0,1 -> merged chunk2 (both b) -> b1 chunks 0,1
            def pool(xt, dst):
                wp = ptmp.tile([128, 1024], F32, tag="wp")
                nc.vector.reduce_sum(
                    out=wp, in_=xt.rearrange("p (a b) -> p a b", b=4), axis=AX.X)
                nc.vector.reduce_sum(
                    out=dst,
                    in_=wp.rearrange("p (hb hi wb) -> p hb wb hi", hi=4, wb=16),
                    axis=AX.X)

            loads = [(0, 0), (0, 1), (None, 2), (1, 0), (1, 1)]
            z0c2 = ptmp.tile([128, N], F32, tag="z0c2")
            for b, k in loads:
                xt = xin.tile([128, 4096], F32, tag="xt")
                if b is not None:
                    nc.sync.dma_start(out=xt, in_=x[b, 128 * k:128 * (k + 1)])
                    pool(xt, z0[:, k, b, :])
                else:
                    nc.sync.dma_start(out=xt, in_=x[:, 256:320])
                    pool(xt, z0c2)
                    nc.sync.dma_start(out=z0[0:64, 2, 0, :], in_=z0c2[0:64, :])
                    nc.sync.dma_start(out=z0[0:64, 2, 1, :], in_=z0c2[64:128, :])

            # padded GN1+SiLU output for conv1, per batch
            def middle(b):
                # ---- GN1 stats ----
                for k in range(3):
                    st6 = ptmp.tile([128, 6], F32, tag="st6")
                    nc.vector.bn_stats(out=st6, in_=z0[:, k, b, :])
                    nc.vector.bn_aggr(out=stat2[:, k, b, :], in_=st6)
                    tm = ptmp.tile([128, 1], F32, tag="tm")
                    nc.vector.tensor_mul(tm, stat2[:, k, b, 0:1], stat2[:, k, b, 0:1])
                    nc.vector.tensor_add(stat2[:, k, b, 1:2], stat2[:, k, b, 1:2], tm)
                pg = psA.tile([32, 2], F32, tag="ps")
                for k in range(3):
                    nc.tensor.matmul(pg, gsumt[:, k, :], stat2[:, k, b, :],
                                     start=(k == 0), stop=(k == 2))
                gm = ptmp.tile([32, 2], F32, tag="gm")
                nc.vector.tensor_copy(gm, pg)
                gv = ptmp.tile([32, 1], F32, tag="gv")
                nc.vector.tensor_mul(gv, gm[:, 0:1], gm[:, 0:1])
                nc.vector.tensor_sub(gv, gm[:, 1:2], gv)
                nc.scalar.activation(out=gv, in_=gv, func=AF.Sqrt, bias=eps1)
                nc.vector.reciprocal(out=ab[:, 2 * b:2 * b + 1], in_=gv)
                nc.vector.tensor_copy(ab[:, 2 * b + 1:2 * b + 2], gm[:, 0:1])
                # ---- broadcast to channels + fold gamma/beta ----
                for k in range(3):
                    pbc = psA.tile([128, 2], F32, tag="ps")
                    nc.tensor.matmul(pbc, gtt[:, k, :], ab[:, 2 * b:2 * b + 2],
                                     start=True, stop=True)
                    # scale = g1 * A
                    nc.vector.tensor_scalar_mul(
                        out=sbias[:, k, b, 0:1], in0=pbc[:, 0:1],
                        scalar1=vcht[:, k, 0:1])
                    tm2 = ptmp.tile([128, 1], F32, tag="tm2")
                    nc.vector.tensor_mul(tm2, sbias[:, k, b, 0:1], pbc[:, 1:2])
                    # bias = b1 - scale*B  (computed as (scale*B - b1) * -1)
                    nc.vector.tensor_scalar(
                        out=sbias[:, k, b, 1:2], in0=tm2,
                        scalar1=vcht[:, k, 1:2], scalar2=-1.0,
                        op0=ALU.subtract, op1=ALU.mult)
                # ---- SiLU into padded tile ----
                pad1 = midb.tile([128, 3, 18, 18], F32, tag="pad1")
                nc.gpsimd.memset(pad1, 0.0)
                for k in range(3):
                    nc.scalar.activation(
                        out=pad1[:, k, 1:17, 1:17],
                        in_=z0[:, k, b, :].rearrange("p (h w) -> p h w", w=16),
                        func=AF.Silu,
                        scale=sbias[:, k, b, 0:1], bias=sbias[:, k, b, 1:2])
                # ---- conv1: 27 accumulating matmuls ----
                pz = psA.tile([8, N], F32, tag="ps")
                first = True
                for k in range(3):
                    for ky in range(3):
                        for kx in range(3):
                            nc.tensor.matmul(
                                pz, w1t[:, k, 3 * ky + kx, :],
                                pad1[:, k, ky:ky + 16, kx:kx + 16],
                                start=first, stop=(k == 2 and ky == 2 and kx == 2))
                            first = False
                nc.vector.tensor_scalar_add(out=zc[:, b, :], in0=pz,
                                            scalar1=c8("b_conv1"))

                # ---- attention ----
                # GN_a (per-channel): stats over 256 pixels
                st6a = ptmp.tile([8, 6], F32, tag="st6a")
                nc.vector.bn_stats(out=st6a, in_=zc[:, b, :])
                mva = ptmp.tile([8, 2], F32, tag="mva")
                nc.vector.bn_aggr(out=mva, in_=st6a)
                ra = ptmp.tile([8, 1], F32, tag="ra")
                nc.scalar.activation(out=ra, in_=mva[:, 1:2], func=AF.Sqrt, bias=eps8)
                nc.vector.reciprocal(out=ra, in_=ra)
                sca = ptmp.tile([8, 2], F32, tag="sca")
                nc.vector.tensor_mul(sca[:, 0:1], c8("ga"), ra)
                tm3 = ptmp.tile([8, 1], F32, tag="tm3")
                nc.vector.tensor_mul(tm3, sca[:, 0:1], mva[:, 0:1])
                nc.vector.tensor_scalar(
                    out=sca[:, 1:2], in0=tm3, scalar1=c8("ba"), scalar2=-1.0,
                    op0=ALU.subtract, op1=ALU.mult)
                zna = ptmp.tile([8, N], F32, tag="zna")
                nc.scalar.activation(out=zna, in_=zc[:, b, :], func=AF.Identity,
                                     scale=sca[:, 0:1], bias=sca[:, 1:2])
                # q/k projections scattered to 32-aligned head bases; v compact
                pq4 = psA.tile([128, N], F32, tag="ps")
                nc.tensor.matmul(pq4, c8("wqS"), zna, start=True, stop=True)
                nc.vector.tensor_scalar_add(out=qs4[:, b, :], in0=pq4, scalar1=bqS_t)
                pk4 = psA.tile([128, N], F32, tag="ps")
                nc.tensor.matmul(pk4, c8("wkS"), zna, start=True, stop=True)
                nc.vector.tensor_scalar_add(out=ks4[:, b, :], in0=pk4, scalar1=bkS_t)
                pv = psA.tile([8, N], F32, tag="ps")
                nc.tensor.matmul(pv, c8("wvT"), zna, start=True, stop=True)
                nc.vector.tensor_scalar_add(out=vsb[:, b, :], in0=pv, scalar1=c8("bv"))

                # v^T per m-chunk (plain matmul against identity: vsb.T @ I)
                for mc in range(2):
                    pvt = psT.tile([128, N], F32, tag="pt")
                    nc.tensor.matmul(pvt[:, 0:8], vsb[:, b, 128 * mc:128 * (mc + 1)],
                                     c8("ident8"), start=True, stop=True)
                    nc.vector.tensor_copy(vT[:, mc, b, :], pvt[:, 0:8])

                # E^T = exp(k^T q); sums at rows 32h (unused rows stay 1.0);
                # EV at rows 32h:32h+2 (unused rows stay 0.0)
                psum4 = psAcc.tile([128, N], F32, tag="psum")
                nc.vector.memset(psum4, 1.0)
                pev4 = psAcc.tile([128, N], F32, tag="pev")
                nc.vector.memset(pev4, 0.0)
                for h in range(4):
                    for mc in range(2):
                        pst = psT.tile([128, N], F32, tag="pt")
                        nc.tensor.matmul(
                            pst,
                            ks4[32 * h:32 * h + 2, b, 128 * mc:128 * (mc + 1)],
                            qs4[32 * h:32 * h + 2, b, :],
                            start=True, stop=True, tile_position=(32 * h, 0))
                        et = etp.tile([128, N], F32, tag="et")
                        nc.scalar.activation(out=et, in_=pst, func=AF.Exp)
                        nc.tensor.matmul(psum4[32 * h:32 * h + 2, :], ones2t, et,
                                         start=(mc == 0), stop=(mc == 1),
                                         tile_position=(0, 32 * h))
                        nc.tensor.matmul(pev4[32 * h:32 * h + 2, :],
                                         vT[:, mc, b, 2 * h:2 * h + 2], et,
                                         start=(mc == 0), stop=(mc == 1),
                                         tile_position=(0, 32 * h))
                den4 = ptmp.tile([128, N], F32, tag="den4")
                nc.vector.reciprocal(out=den4, in_=psum4)
                ao4 = ptmp.tile([128, N], F32, tag="ao4")
                nc.vector.tensor_mul(ao4, pev4, den4)
                # gather heads + project in one matmul
                pp = psA.tile([8, N], F32, tag="ps")
                nc.tensor.matmul(pp, wpS_t, ao4, start=True, stop=True)
                nc.vector.tensor_scalar_add(out=z1[:, b, :], in0=pp,
                                            scalar1=c8("b_proj"))
                nc.vector.tensor_add(z1[:, b, :], z1[:, b, :], zc[:, b, :])

                # ---- per-pixel LN + MLP ----
                sq8 = ptmp.tile([8, N], F32, tag="sq8")
                nc.vector.tensor_mul(sq8, z1[:, b, :], z1[:, b, :])
                pln = psA.tile([33, N], F32, tag="ps")
                nc.tensor.matmul(pln[0:1, :], c8("ones8"), z1[:, b, :],
                                 start=True, stop=True)
                nc.tensor.matmul(pln[32:33, :], c8("ones8"), sq8,
                                 start=True, stop=True)
                murs = ptmp.tile([1, 2, N], F32, tag="murs")
                nc.scalar.mul(out=murs[:, 0, :], in_=pln[0:1, :], mul=1.0 / 8)
                ex2 = ptmp.tile([1, N], F32, tag="ex2")
                nc.scalar.mul(out=ex2, in_=pln[32:33, :], mul=1.0 / 8)
                musq = ptmp.tile([1, N], F32, tag="musq")
                nc.vector.tensor_mul(musq, murs[:, 0, :], murs[:, 0, :])
                nc.vector.tensor_sub(ex2, ex2, musq)
                nc.scalar.activation(out=ex2, in_=ex2, func=AF.Sqrt, bias=eps1p)
                nc.vector.reciprocal(out=murs[:, 1, :], in_=ex2)
                pbr = psA.tile([8, 2, N], F32, tag="ps")
                nc.tensor.matmul(pbr.rearrange("p a b -> p (a b)"), one18t,
                                 murs.rearrange("p a b -> p (a b)"),
                                 start=True, stop=True)
                cen = ptmp.tile([8, N], F32, tag="cen")
                nc.vector.tensor_sub(cen, z1[:, b, :], pbr[:, 0, :])
                nc.vector.tensor_mul(cen, cen, pbr[:, 1, :])
                lnt = ptmp.tile([8, N], F32, tag="lnt")
                nc.vector.tensor_scalar(
                    out=lnt, in0=cen, scalar1=c8("ln_g"), scalar2=c8("ln_b"),
                    op0=ALU.mult, op1=ALU.add)
                pf1 = psA.tile([16, N], F32, tag="ps")
                nc.tensor.matmul(pf1, c8("wf1T"), lnt, start=True, stop=True)
                hmid = ptmp.tile([16, N], F32, tag="hmid")
                nc.scalar.activation(out=hmid, in_=pf1, func=AF.Gelu, bias=bf1)
                pf2 = psA.tile([8, N], F32, tag="ps")
                nc.tensor.matmul(pf2, wf2T, hmid, start=True, stop=True)
                nc.vector.tensor_scalar_add(out=z2[:, b, :], in0=pf2,
                                            scalar1=c8("b_fc2"))
                nc.vector.tensor_add(z2[:, b, :], z2[:, b, :], z1[:, b, :])

                # ---- GN2 + SiLU into padded tile ----
                st6b = ptmp.tile([8, 6], F32, tag="st6b")
                nc.vector.bn_stats(out=st6b, in_=z2[:, b, :])
                mvb = ptmp.tile([8, 2], F32, tag="mvb")
                nc.vector.bn_aggr(out=mvb, in_=st6b)
                rb2 = ptmp.tile([8, 1], F32, tag="rb2")
                nc.scalar.activation(out=rb2, in_=mvb[:, 1:2], func=AF.Sqrt, bias=eps8)
                nc.vector.reciprocal(out=rb2, in_=rb2)
                scb = ptmp.tile([8, 2], F32, tag="scb")
                nc.vector.tensor_mul(scb[:, 0:1], c8("g2"), rb2)
                tm4 = ptmp.tile([8, 1], F32, tag="tm4")
                nc.vector.tensor_mul(tm4, scb[:, 0:1], mvb[:, 0:1])
                nc.vector.tensor_scalar(
                    out=scb[:, 1:2], in0=tm4, scalar1=c8("b2"), scalar2=-1.0,
                    op0=ALU.subtract, op1=ALU.mult)
                spad = midb.tile([8, 18, 18], F32, tag="spad")
                nc.gpsimd.memset(spad, 0.0)
                nc.scalar.activation(
                    out=spad[:, 1:17, 1:17],
                    in_=z2[:, b, :].rearrange("p (h w) -> p h w", w=16),
                    func=AF.Silu, scale=scb[:, 0:1], bias=scb[:, 1:2])
                # 9-slot shifted-window stack
                sp9 = midb.tile([72, N], F32, tag="sp9")
                for dy in range(3):
                    for dx in range(3):
                        slot = 3 * dy + dx
                        nc.sync.dma_start(
                            out=sp9[8 * slot:8 * slot + 8, :],
                            in_=spad[:, dy:dy + 16, dx:dx + 16])
                return sp9

            sp9s = []
            for b in range(B):
                sp9s.append(middle(b))

            # ---------------- phase 7: conv2 + interleave + out ----------------
            for c in range(3):
                if c < 2:
                    per_b_tiles = [
                        outp.tile([128, 64, 64], F32, tag="oc",
                                  name=f"oc_c{c}_b{bb}") for bb in range(B)]
                else:
                    shared = outp.tile([128, 64, 64], F32, tag="oc",
                                       name="oc_c2")
                    per_b_tiles = [shared, shared]
                for g, (pi, pj) in enumerate(GROUPS):
                    r0, nr = ROWSETS[pi]
                    c0, ncc = ROWSETS[pj]
                    for b in range(B):
                        if c < 2:
                            pcv = psC.tile([128, 16, 16], F32, tag="pcv")
                            mm_out = pcv
                            p0, pn = 0, 128
                        else:
                            pcv = psC.tile([128, 16, 16], F32, tag="pcv")
                            p0, pn = 64 * b, 64
                            mm_out = pcv[p0:p0 + 64]
                        nc.tensor.matmul(
                            mm_out.rearrange("p a b -> p (a b)"),
                            w9t[:, g, c, p0:p0 + pn],
                            sp9s[b][:, :],
                            start=True, stop=True)
                        # ISA allows at most 3 free dims, so emit one copy per
                        # duplicated row; columns duplicate via a step-0 dim.
                        ot = per_b_tiles[b]
                        base5 = ot.rearrange(
                            "p (bi ri) (bj rj) -> p bi ri bj rj", ri=4, rj=4)
                        src = mm_out.unsqueeze(3).broadcast_to(
                            [pn, 16, 16, ncc])
                        bias_ap = vcht[p0:p0 + pn, c, 2:3]
                        for rr in range(nr):
                            dst = base5[p0:p0 + pn, :, r0 + rr, :, c0:c0 + ncc]
                            if (g + c + rr) % 2 == 0:
                                nc.vector.tensor_scalar_add(out=dst, in0=src,
                                                            scalar1=bias_ap)
                            else:
                                nc.scalar.activation(out=dst, in_=src,
                                                     func=AF.Identity,
                                                     bias=bias_ap)
                # output DMA
                if c < 2:
                    for b in range(B):
                        nc.sync.dma_start(
                            out=out[b, 128 * c:128 * (c + 1)],
                            in_=per_b_tiles[b])
                else:
                    nc.sync.dma_start(out=out[0, 256:320], in_=shared[0:64])
                    nc.sync.dma_start(out=out[1, 256:320], in_=shared[64:128])
    nc.compile()
    return nc


_cache = {}


def kernel(**inputs):
    x = np.ascontiguousarray(np.asarray(inputs["x"], np.float32))
    params = {k: np.asarray(v, np.float32) for k, v in inputs.items() if k != "x"}

    key = tuple(sorted((k, v.tobytes()) for k, v in params.items()))
    key = hash(key)
    if key not in _cache:
        _cache[key] = build(params)
    nc = _cache[key]

    in_maps = [{"x": np.ascontiguousarray(x[B * i:B * (i + 1)])}
               for i in range(NCORES)]
    res = run_bass_kernel_spmd(nc, in_maps, core_ids=list(range(NCORES)),
                               trace=KERNEL_TRACE)
    out = np.concatenate([res.results[i]["out"] for i in range(NCORES)], axis=0)
    if KERNEL_TRACE:
        kernel.last_result = res
    return out


# revision 17
# speedup vs baseline: 1.8222x; 1.8222x over previous
"""Trainium2 Bass kernel for the nn_Adaptor problem.

Computation (per batch image):
  avgpool4x4 -> GN(32 groups)+SiLU -> conv3x3 320->8 -> attention(4 heads) ->
  per-pixel LN + MLP -> GN(8)+SiLU -> upsample x4 nearest -> conv3x3 8->320

Distribution: pure data parallel over batch. 16 images / 8 cores = 2 per core.
Params are baked into the NEFF as inline consts (recomputed from the numpy
arrays passed to kernel() at trace time).

Implementation notes:
  - pooling keeps raw 4x4 sums (16x scale); GN1 uses eps_eff = 256*eps so the
    normalized output is exact.
  - GN1 group stats via per-channel bn_stats + grouping-matrix matmuls on PE.
  - All norm+SiLU applications fused into single scalar-engine activations.
  - conv1 as 9 shifted-window matmuls over a zero-padded 18x18 tile; both
    local batch images stacked along the matmul free dim (N=512).
  - attention: transposed scores E^T = exp(k^T q) without max subtraction
    (|scores| < 0.5 for this operator family); two heads per matmul via
    zero-masked q blocks; softmax denominators via ones-matmul column sums;
    head gather folded into zero-masked projection matmuls.
  - GN2 applied pre-upsample (nearest-upsample preserves per-channel stats).
  - conv2-after-upsample collapses to 9 phase groups with collapsed weights on
    the 16x16 grid (K=72 over a 9-slot shifted-window stack); phase outputs are
    interleaved on-chip by strided copies with step-0 column duplication, then
    written out with fully contiguous DMAs split across both HWDGE rings.
  - the middle is one batch-stacked dependency chain; engine streams execute
    in order, so fewer/wider ops beat two interleaved per-batch chains.
"""

import ml_dtypes
import numpy as np

import concourse.bass as bass
import concourse.bacc as bacc
import concourse.tile as tile
from concourse import mybir
from concourse.bass_utils import run_bass_kernel_spmd

F32 = mybir.dt.float32
BF16 = mybir.dt.bfloat16
NPBF = ml_dtypes.bfloat16
AF = mybir.ActivationFunctionType
ALU = mybir.AluOpType
AX = mybir.AxisListType

CH, C, D, HEADS = 320, 4, 8, 4
EPS = 1e-5
B = 2
N = 256
NCORES = 8

TAPSETS = {
    0: [(0, (0,)), (1, (1, 2))],
    1: [(1, (0, 1, 2))],
    3: [(1, (0, 1)), (2, (2,))],
}
GROUPS = [(pi, pj) for pi in (0, 1, 3) for pj in (0, 1, 3)]
ROWSETS = {0: (0, 1), 1: (1, 2), 3: (3, 1)}   # (start row, duplication count)

KERNEL_TRACE = False


def _conv2_tables(w_conv2):
    """W9 [72, 9, 3, 128]: collapsed per-phase-group weights over the 9-slot
    shifted-window stack; chunk 2 duplicated into cols 64:128 (two-batch
    chunk-2 matmul keeps batch 1 at psum partitions 64:128)."""
    W9 = np.zeros((72, 9, 3, 128), np.float32)
    for g, (pi, pj) in enumerate(GROUPS):
        for (dy, kys) in TAPSETS[pi]:
            for (dx, kxs) in TAPSETS[pj]:
                s = 3 * dy + dx
                wsum = np.zeros((CH, D), np.float32)
                for ky in kys:
                    for kx in kxs:
                        wsum += w_conv2[:, :, ky, kx]
                for c in range(3):
                    oc0 = 128 * c
                    ocn = min(128, CH - oc0)
                    blk = wsum[oc0:oc0 + ocn].T
                    W9[8 * s:8 * s + 8, g, c, :ocn] += blk
                    if c == 2:
                        W9[8 * s:8 * s + 8, g, c, 64:64 + ocn] += blk
    return W9


def build(params):
    P = params
    nc = bacc.Bacc("TRN2")

    x = nc.dram_tensor("x", [B, CH, 64, 64], F32, kind="ExternalInput")
    out = nc.dram_tensor("out", [B, CH, 64, 64], F32, kind="ExternalOutput")

    # ---------------- host-side constant prep ----------------
    s = float(1 / np.sqrt(D // HEADS))
    wq = P["w_qkv"].copy()
    bq = P["b_qkv"].copy()
    wq[D:2 * D] *= s
    bq[D:2 * D] *= s

    W1 = np.zeros((128, 3, 9, 8), np.float32)
    for c in range(3):
        c0 = 128 * c
        cn = min(128, CH - c0)
        for ky in range(3):
            for kx in range(3):
                W1[:cn, c, 3 * ky + kx, :] = P["w_conv1"][:, c0:c0 + cn, ky, kx].T

    W9 = _conv2_tables(P["w_conv2"])

    Gsum = np.zeros((128, 3, 32), np.float32)
    GT = np.zeros((32, 3, 128), np.float32)
    for c in range(CH):
        k, p = divmod(c, 128)
        Gsum[p, k, c // 10] = 0.1
        GT[c // 10, k, p] = 1.0

    def chunks(v, dup2=False):
        a = np.zeros((128, 3), np.float32)
        for c in range(3):
            c0 = 128 * c
            cn = min(128, CH - c0)
            a[:cn, c] = v[c0:c0 + cn]
            if dup2 and c == 2:
                a[64:64 + cn, c] = v[c0:c0 + cn]
        return a

    vch = np.stack([chunks(P["g1"]), chunks(P["b1"]),
                    chunks(P["b_conv2"], dup2=True)], axis=2)  # [128, 3, 3]

    cols8, pk8 = {}, []

    def pack8(name, arr):
        arr = np.asarray(arr, np.float32).reshape(8, -1)
        cols8[name] = (sum(a.shape[1] for a in pk8), arr.shape[1])
        pk8.append(arr)

    pack8("ones8", np.ones((8, 1), np.float32))
    for nm, val in [("bq8", bq[0:D]), ("bk8", bq[D:2 * D]), ("bv", bq[2 * D:]),
                    ("b_conv1", P["b_conv1"]), ("b_proj", P["b_proj"]),
                    ("ln_g", P["ln_g"]), ("ln_b", P["ln_b"]),
                    ("b_fc2", P["b_fc2"]), ("ga", P["ga"]), ("ba", P["ba"]),
                    ("g2", P["g2"]), ("b2", P["b2"])]:
        pack8(nm, val.reshape(8, 1))
    PK8 = np.concatenate(pk8, axis=1)

    cols8b, pk8b = {}, []

    def pack8b(name, arr):
        arr = np.asarray(arr, np.float32).reshape(8, -1)
        cols8b[name] = (sum(a.shape[1] for a in pk8b), arr.shape[1])
        pk8b.append(arr)

    pack8b("wqT", wq[0:D].T)
    pack8b("wkT", wq[D:2 * D].T)
    pack8b("wvT", P["w_qkv"][2 * D:3 * D].T)
    pack8b("ident8", np.eye(8, dtype=np.float32))
    qmask = np.zeros((8, 4), np.float32)
    for c in range(8):
        qmask[c, c // 2] = 1.0
    pack8b("qmask", qmask)
    wpH = np.zeros((8, 4, 8), np.float32)
    for c in range(8):
        wpH[c, c // 2, :] = P["w_proj"][:, c]
    pack8b("wpH", wpH.reshape(8, 32))
    pack8b("wf1T", P["w_fc1"].T)
    PK8B = np.concatenate(pk8b, axis=1).astype(NPBF)

    PK16 = np.concatenate([P["w_fc2"].T, P["b_fc1"].reshape(16, 1)], axis=1)
    WF2B = P["w_fc2"].T.astype(NPBF)

    h_w1 = nc.inline_tensor(W1.astype(NPBF), "cW1")
    h_w9 = nc.inline_tensor(W9.astype(NPBF), "cW9")
    h_gsum = nc.inline_tensor(Gsum, "cGsum")
    h_gt = nc.inline_tensor(GT, "cGT")
    h_vch = nc.inline_tensor(vch, "cVch")
    h_pk8 = nc.inline_tensor(PK8, "cPK8")
    h_pk8b = nc.inline_tensor(PK8B, "cPK8B")
    h_pk16 = nc.inline_tensor(PK16, "cPK16")
    h_wf2b = nc.inline_tensor(WF2B, "cWF2B")
    h_one18 = nc.inline_tensor(np.ones((1, 8), np.float32), "cOne18")
    h_ones128 = nc.inline_tensor(
        np.ones((128, 8), np.float32).astype(NPBF), "cOnes128")

    with tile.TileContext(nc) as tc:
        with (
            tc.tile_pool(name="consts", bufs=1) as csts,
            tc.tile_pool(name="xin", bufs=3) as xin,
            tc.tile_pool(name="pooltmp", bufs=2) as ptmp,
            tc.tile_pool(name="mid", bufs=1) as mid,
            tc.tile_pool(name="et", bufs=4) as etp,
            tc.tile_pool(name="outp", bufs=4) as outp,
            tc.tile_pool(name="psA", bufs=2, space="PSUM") as psA,
            tc.tile_pool(name="psT", bufs=2, space="PSUM") as psT,
            tc.tile_pool(name="psAcc", bufs=2, space="PSUM") as psAcc,
            tc.tile_pool(name="psC", bufs=2, space="PSUM") as psC,
        ):
            # ---------------- consts ----------------
            w1t = csts.tile([128, 3, 9, 8], BF16)
            nc.gpsimd.dma_start(out=w1t, in_=h_w1[:])
            w9t = csts.tile([72, 9, 3, 128], BF16)
            nc.gpsimd.dma_start(out=w9t, in_=h_w9[:])
            gsumt = csts.tile([128, 3, 32], F32)
            nc.gpsimd.dma_start(out=gsumt, in_=h_gsum[:])
            gtt = csts.tile([32, 3, 128], F32)
            nc.gpsimd.dma_start(out=gtt, in_=h_gt[:])
            vcht = csts.tile([128, 3, 3], F32)
            nc.gpsimd.dma_start(out=vcht, in_=h_vch[:])
            pk8t = csts.tile([8, PK8.shape[1]], F32)
            nc.gpsimd.dma_start(out=pk8t, in_=h_pk8[:])
            pk8bt = csts.tile([8, PK8B.shape[1]], BF16)
            nc.gpsimd.dma_start(out=pk8bt, in_=h_pk8b[:])
            pk16t = csts.tile([16, 9], F32)
            nc.gpsimd.dma_start(out=pk16t, in_=h_pk16[:])
            wf2bt = csts.tile([16, 8], BF16)
            nc.gpsimd.dma_start(out=wf2bt, in_=h_wf2b[:])
            one18t = csts.tile([1, 8], F32)
            nc.gpsimd.dma_start(out=one18t, in_=h_one18[:])
            ones8x = csts.tile([128, 8], BF16)
            nc.gpsimd.dma_start(out=ones8x, in_=h_ones128[:])

            def c8(name):
                c0, w = cols8[name]
                return pk8t[:, c0:c0 + w]

            def c8b(name):
                c0, w = cols8b[name]
                return pk8bt[:, c0:c0 + w]

            bf1 = pk16t[:, 8:9]

            eps1 = csts.tile([32, 1], F32)
            nc.vector.memset(eps1, 256.0 * EPS)
            eps8 = csts.tile([8, 1], F32)
            nc.vector.memset(eps8, EPS)
            eps1p = csts.tile([1, 1], F32)
            nc.vector.memset(eps1p, EPS)

            # ---------------- state ----------------
            z0 = mid.tile([128, 3, B, N], F32)
            nc.vector.memset(z0[64:128, 2, :, :], 0.0)
            stat2 = mid.tile([128, 3, B, 2], F32)
            ab = mid.tile([32, 2 * B], F32)
            sbias = mid.tile([128, 3, B, 2], F32)
            zc = mid.tile([8, B, N], F32)
            zna = mid.tile([8, B, N], BF16)
            qsb = mid.tile([8, B, N], BF16)
            ksb = mid.tile([8, B, N], BF16)
            vsb = mid.tile([8, B, N], BF16)
            qblk = mid.tile([8, B, 2, 2, N], BF16)
            vT = mid.tile([128, 2, B, 8], BF16)
            z1 = mid.tile([8, B, N], F32)
            z2 = mid.tile([8, B, N], F32)
            pad1 = mid.tile([128, 3, B, 18, 18], BF16)
            spad = mid.tile([8, B, 18, 18], BF16)
            sp9 = mid.tile([72, B, N], BF16)

            # ---------------- phase 1: load + pool ----------------
            def pool(xt, dst):
                wp = ptmp.tile([128, 1024], F32, tag="wp")
                nc.vector.reduce_sum(
                    out=wp, in_=xt.rearrange("p (a b) -> p a b", b=4), axis=AX.X)
                wpv = wp.rearrange("p (hb hi wb) -> p hb hi wb", hi=4, wb=16)
                t01 = ptmp.tile([128, 16, 16], F32, tag="t01")
                nc.gpsimd.tensor_add(t01, wpv[:, :, 0, :], wpv[:, :, 1, :])
                t23 = ptmp.tile([128, 16, 16], F32, tag="t23")
                nc.gpsimd.tensor_add(t23, wpv[:, :, 2, :], wpv[:, :, 3, :])
                nc.gpsimd.tensor_add(dst, t01, t23)

            loads = [(0, 0), (0, 1), (None, 2), (1, 0), (1, 1)]
            z0c2 = ptmp.tile([128, N], F32, tag="z0c2")
            with nc.named_scope("pool"):
                for b, k in loads:
                    xt = xin.tile([128, 4096], F32, tag="xt")
                    if b is not None:
                        src_ap = x[b, 128 * k:128 * (k + 1)].rearrange(
                            "c h w -> c (h w)")
                        nc.sync.dma_start(out=xt[:, 0:2048], in_=src_ap[:, 0:2048])
                        nc.scalar.dma_start(out=xt[:, 2048:4096],
                                            in_=src_ap[:, 2048:4096])
                        pool(xt, z0[:, k, b, :])
                    else:
                        for bb in range(2):
                            src_ap = x[bb, 256:320].rearrange("c h w -> c (h w)")
                            eng = nc.sync if bb == 0 else nc.scalar
                            eng.dma_start(out=xt[64 * bb:64 * bb + 64, :],
                                          in_=src_ap)
                        pool(xt, z0c2)
                        nc.gpsimd.dma_start(out=z0[0:64, 2, 0, :],
                                            in_=z0c2[0:64, :])
                        nc.gpsimd.dma_start(out=z0[0:64, 2, 1, :],
                                            in_=z0c2[64:128, :])

            # ---------------- middle: one batch-stacked chain ----------------
            with nc.named_scope("middle"):
                # GN1 per-channel stats
                for k in range(3):
                    for b in range(B):
                        st6 = ptmp.tile([128, 6], F32, tag="st6")
                        nc.vector.bn_stats(out=st6, in_=z0[:, k, b, :])
                        nc.vector.bn_aggr(out=stat2[:, k, b, :], in_=st6)
                    tm = ptmp.tile([128, 2], F32, tag="tm")
                    nc.vector.tensor_mul(tm, stat2[:, k, :, 0], stat2[:, k, :, 0])
                    nc.vector.tensor_add(stat2[:, k, :, 1], stat2[:, k, :, 1], tm)
                pg = psA.tile([32, 4], F32, tag="ps")
                for k in range(3):
                    nc.tensor.matmul(
                        pg, gsumt[:, k, :],
                        stat2[:, k, :, :].rearrange("p a b -> p (a b)"),
                        start=(k == 0), stop=(k == 2))
                gm = ptmp.tile([32, 4], F32, tag="gm")
                nc.vector.tensor_copy(gm, pg)
                gv = ptmp.tile([32, 2], F32, tag="gv")
                nc.vector.tensor_mul(gv, gm[:, 0::2], gm[:, 0::2])
                nc.vector.tensor_sub(gv, gm[:, 1::2], gv)
                nc.scalar.activation(out=gv, in_=gv, func=AF.Sqrt, bias=eps1)
                nc.vector.reciprocal(out=ab[:, 0::2], in_=gv)
                nc.vector.tensor_copy(ab[:, 1::2], gm[:, 0::2])
                for k in range(3):
                    pbc = psA.tile([128, 4], F32, tag="ps")
                    nc.tensor.matmul(pbc, gtt[:, k, :], ab,
                                     start=True, stop=True)
                    nc.vector.tensor_scalar_mul(
                        out=sbias[:, k, :, 0], in0=pbc[:, 0::2],
                        scalar1=vcht[:, k, 0:1])
                    tm2 = ptmp.tile([128, 2], F32, tag="tm2")
                    nc.vector.tensor_mul(tm2, sbias[:, k, :, 0], pbc[:, 1::2])
                    nc.vector.tensor_scalar(
                        out=sbias[:, k, :, 1], in0=tm2,
                        scalar1=vcht[:, k, 1:2], scalar2=-1.0,
                        op0=ALU.subtract, op1=ALU.mult)
                nc.gpsimd.memset(pad1, 0.0)
                for k in range(3):
                    for b in range(B):
                        nc.scalar.activation(
                            out=pad1[:, k, b, 1:17, 1:17],
                            in_=z0[:, k, b, :].rearrange("p (h w) -> p h w", w=16),
                            func=AF.Silu,
                            scale=sbias[:, k, b, 0:1], bias=sbias[:, k, b, 1:2])
                # conv1 (27 accumulating matmuls, both batches in N)
                pz = psA.tile([8, B, N], F32, tag="ps")
                first = True
                for k in range(3):
                    for ky in range(3):
                        for kx in range(3):
                            nc.tensor.matmul(
                                pz.rearrange("p a b -> p (a b)"),
                                w1t[:, k, 3 * ky + kx, :],
                                pad1[:, k, :, ky:ky + 16, kx:kx + 16],
                                start=first,
                                stop=(k == 2 and ky == 2 and kx == 2))
                            first = False
                nc.vector.tensor_scalar_add(out=zc, in0=pz,
                                            scalar1=c8("b_conv1"))

                # ---- attention ----
                st6a = ptmp.tile([8, B, 6], F32, tag="st6a")
                mva = ptmp.tile([8, B, 2], F32, tag="mva")
                for b in range(B):
                    nc.vector.bn_stats(out=st6a[:, b, :], in_=zc[:, b, :])
                    nc.vector.bn_aggr(out=mva[:, b, :], in_=st6a[:, b, :])
                ra = ptmp.tile([8, B], F32, tag="ra")
                nc.scalar.activation(out=ra, in_=mva[:, :, 1], func=AF.Sqrt,
                                     bias=eps8)
                nc.vector.reciprocal(out=ra, in_=ra)
                sca = ptmp.tile([8, B, 2], F32, tag="sca")
                nc.vector.tensor_scalar_mul(out=sca[:, :, 0], in0=ra,
                                            scalar1=c8("ga"))
                tm3 = ptmp.tile([8, B], F32, tag="tm3")
                nc.vector.tensor_mul(tm3, sca[:, :, 0], mva[:, :, 0])
                nc.vector.tensor_scalar(
                    out=sca[:, :, 1], in0=tm3, scalar1=c8("ba"), scalar2=-1.0,
                    op0=ALU.subtract, op1=ALU.mult)
                for b in range(B):
                    nc.scalar.activation(out=zna[:, b, :], in_=zc[:, b, :],
                                         func=AF.Identity,
                                         scale=sca[:, b, 0:1],
                                         bias=sca[:, b, 1:2])
                znaf = zna.rearrange("p a b -> p (a b)")
                for wname, bname, dst in [("wqT", "bq8", qsb),
                                          ("wkT", "bk8", ksb),
                                          ("wvT", "bv", vsb)]:
                    pqkv = psA.tile([8, B, N], F32, tag="ps",
                                    name=f"pqkv_{wname}")
                    nc.tensor.matmul(pqkv.rearrange("p a b -> p (a b)"),
                                     c8b(wname), znaf, start=True, stop=True)
                    nc.vector.tensor_scalar_add(out=dst, in0=pqkv,
                                                scalar1=c8(bname))
                qmt = c8b("qmask").rearrange("p (a c) -> p a c", a=2)
                for b in range(B):
                    nc.vector.tensor_mul(
                        qblk[:, b],
                        qsb[:, b, :].unsqueeze(1).unsqueeze(1).broadcast_to(
                            [8, 2, 2, N]),
                        qmt.unsqueeze(3).broadcast_to([8, 2, 2, N]))
                for b in range(B):
                    for mc in range(2):
                        pvt = psT.tile([128, 2, N], F32, tag="pt")
                        nc.tensor.matmul(pvt[:, 0, 0:8],
                                         vsb[:, b, 128 * mc:128 * (mc + 1)],
                                         c8b("ident8"), start=True, stop=True)
                        nc.vector.tensor_copy(vT[:, mc, b, :], pvt[:, 0, 0:8])
                wpH_t = c8b("wpH").rearrange("p (a c) -> p a c", a=4)
                pp = psA.tile([8, B, N], F32, tag="ps")
                for b in range(B):
                    for blk in range(2):
                        psum_s = psAcc.tile([8, 2, N], F32, tag="acc",
                                            name=f"psum_s{b}{blk}")
                        psum_e = psAcc.tile([8, 2, N], F32, tag="acc",
                                            name=f"psum_e{b}{blk}")
                        for mc in range(2):
                            pst = psT.tile([128, 2, N], F32, tag="pt")
                            nc.tensor.matmul(
                                pst.rearrange("p a n -> p (a n)"),
                                ksb[:, b, 128 * mc:128 * (mc + 1)],
                                qblk[:, b, blk].rearrange("p a n -> p (a n)"),
                                start=True, stop=True)
                            et = etp.tile([128, 2, N], BF16, tag="et")
                            nc.scalar.activation(out=et, in_=pst, func=AF.Exp)
                            etf = et.rearrange("p a n -> p (a n)")
                            nc.tensor.matmul(
                                psum_s.rearrange("p a n -> p (a n)"),
                                ones8x, etf, start=(mc == 0), stop=(mc == 1))
                            nc.tensor.matmul(
                                psum_e.rearrange("p a n -> p (a n)"),
                                vT[:, mc, b, :], etf,
                                start=(mc == 0), stop=(mc == 1))
                        den = ptmp.tile([8, 2, N], F32, tag="den")
                        nc.vector.reciprocal(out=den, in_=psum_s)
                        aoblk = ptmp.tile([8, 2, N], BF16, tag="aoblk")
                        nc.vector.tensor_mul(aoblk, psum_e, den)
                        # cross-head lanes: finite garbage x zero proj weight
                        for hp in range(2):
                            nc.tensor.matmul(pp[:, b, :],
                                             wpH_t[:, 2 * blk + hp, :],
                                             aoblk[:, hp, :],
                                             start=(blk == 0 and hp == 0),
                                             stop=(blk == 1 and hp == 1))
                nc.vector.tensor_scalar_add(out=z1, in0=pp, scalar1=c8("b_proj"))
                nc.vector.tensor_add(z1, z1, zc)

                # ---- per-pixel LN + MLP (batch-stacked) ----
                z1f = z1.rearrange("p a b -> p (a b)")
                sq8 = ptmp.tile([8, B, N], F32, tag="sq8")
                nc.vector.tensor_mul(sq8, z1, z1)
                pln = psA.tile([33, B, N], F32, tag="ps")
                nc.tensor.matmul(pln[0:1, :, :].rearrange("p a b -> p (a b)"),
                                 c8("ones8"), z1f, start=True, stop=True)
                nc.tensor.matmul(pln[32:33, :, :].rearrange("p a b -> p (a b)"),
                                 c8("ones8"), sq8.rearrange("p a b -> p (a b)"),
                                 start=True, stop=True)
                murs = ptmp.tile([1, 2, B, N], F32, tag="murs")
                nc.scalar.mul(out=murs[:, 0], in_=pln[0:1, :, :], mul=1.0 / 8)
                ex2 = ptmp.tile([1, B, N], F32, tag="ex2")
                nc.scalar.mul(out=ex2, in_=pln[32:33, :, :], mul=1.0 / 8)
                musq = ptmp.tile([1, B, N], F32, tag="musq")
                nc.vector.tensor_mul(musq, murs[:, 0], murs[:, 0])
                nc.vector.tensor_sub(ex2, ex2, musq)
                nc.scalar.activation(out=ex2, in_=ex2, func=AF.Sqrt, bias=eps1p)
                nc.vector.reciprocal(out=murs[:, 1], in_=ex2)
                pbr_mu = psA.tile([8, B, N], F32, tag="ps", name="pbr_mu")
                nc.tensor.matmul(pbr_mu.rearrange("p a b -> p (a b)"), one18t,
                                 murs[:, 0].rearrange("p a b -> p (a b)"),
                                 start=True, stop=True)
                pbr_rs = psA.tile([8, B, N], F32, tag="ps", name="pbr_rs")
                nc.tensor.matmul(pbr_rs.rearrange("p a b -> p (a b)"), one18t,
                                 murs[:, 1].rearrange("p a b -> p (a b)"),
                                 start=True, stop=True)
                cen = ptmp.tile([8, B, N], F32, tag="cen")
                nc.vector.tensor_sub(cen, z1, pbr_mu)
                nc.vector.tensor_mul(cen, cen, pbr_rs)
                lnt = ptmp.tile([8, B, N], BF16, tag="lnt")
                nc.vector.tensor_scalar(
                    out=lnt, in0=cen, scalar1=c8("ln_g"), scalar2=c8("ln_b"),
                    op0=ALU.mult, op1=ALU.add)
                pf1 = psA.tile([16, B, N], F32, tag="ps")
                nc.tensor.matmul(pf1.rearrange("p a b -> p (a b)"), c8b("wf1T"),
                                 lnt.rearrange("p a b -> p (a b)"),
                                 start=True, stop=True)
                hmid = ptmp.tile([16, B, N], BF16, tag="hmid")
                nc.scalar.activation(out=hmid, in_=pf1, func=AF.Gelu, bias=bf1)
                pf2 = psA.tile([8, B, N], F32, tag="ps")
                nc.tensor.matmul(pf2.rearrange("p a b -> p (a b)"), wf2bt,
                                 hmid.rearrange("p a b -> p (a b)"),
                                 start=True, stop=True)
                nc.vector.tensor_scalar_add(out=z2, in0=pf2, scalar1=c8("b_fc2"))
                nc.vector.tensor_add(z2, z2, z1)

                # ---- GN2 + SiLU into padded tile ----
                st6b = ptmp.tile([8, B, 6], F32, tag="st6b")
                mvb = ptmp.tile([8, B, 2], F32, tag="mvb")
                for b in range(B):
                    nc.vector.bn_stats(out=st6b[:, b, :], in_=z2[:, b, :])
                    nc.vector.bn_aggr(out=mvb[:, b, :], in_=st6b[:, b, :])
                rb2 = ptmp.tile([8, B], F32, tag="rb2")
                nc.scalar.activation(out=rb2, in_=mvb[:, :, 1], func=AF.Sqrt,
                                     bias=eps8)
                nc.vector.reciprocal(out=rb2, in_=rb2)
                scb = ptmp.tile([8, B, 2], F32, tag="scb")
                nc.vector.tensor_scalar_mul(out=scb[:, :, 0], in0=rb2,
                                            scalar1=c8("g2"))
                tm4 = ptmp.tile([8, B], F32, tag="tm4")
                nc.vector.tensor_mul(tm4, scb[:, :, 0], mvb[:, :, 0])
                nc.vector.tensor_scalar(
                    out=scb[:, :, 1], in0=tm4, scalar1=c8("b2"), scalar2=-1.0,
                    op0=ALU.subtract, op1=ALU.mult)
                nc.gpsimd.memset(spad, 0.0)
                for b in range(B):
                    nc.scalar.activation(
                        out=spad[:, b, 1:17, 1:17],
                        in_=z2[:, b, :].rearrange("p (h w) -> p h w", w=16),
                        func=AF.Silu, scale=scb[:, b, 0:1], bias=scb[:, b, 1:2])
                for dy in range(3):
                    for dx in range(3):
                        slot = 3 * dy + dx
                        for b in range(B):
                            nc.gpsimd.dma_start(
                                out=sp9[8 * slot:8 * slot + 8, b, :],
                                in_=spad[:, b, dy:dy + 16, dx:dx + 16])

            # ---------------- conv2 + interleave + out ----------------
            sp9f = sp9.rearrange("p a b -> p (a b)")
            with nc.named_scope("conv2"):
                for c in range(3):
                    if c < 2:
                        ots = [outp.tile([128, 64, 64], F32, tag="oc",
                                         name=f"oc_b{bb}c{c}")
                               for bb in range(B)]
                    else:
                        shared = outp.tile([128, 64, 64], F32, tag="oc",
                                           name="oc_c2")
                        ots = [shared, shared]
                    for g, (pi, pj) in enumerate(GROUPS):
                        r0, nr = ROWSETS[pi]
                        c0, ncc = ROWSETS[pj]
                        pcv = psC.tile([128, B, 16, 16], F32, tag="pcv")
                        nc.tensor.matmul(
                            pcv.rearrange("p a b c -> p (a b c)"),
                            w9t[:, g, c, :], sp9f, start=True, stop=True)
                        ncopy = 0
                        for b in range(B):
                            if c < 2:
                                p0, pn = 0, 128
                            else:
                                p0, pn = 64 * b, 64
                            src_b = pcv[p0:p0 + pn, b].unsqueeze(3).broadcast_to(
                                [pn, 16, 16, ncc])
                            base5 = ots[b].rearrange(
                                "p (bi ri) (bj rj) -> p bi ri bj rj",
                                ri=4, rj=4)
                            bias_ap = vcht[p0:p0 + pn, c, 2:3]
                            for rr in range(nr):
                                dst = base5[p0:p0 + pn, :, r0 + rr, :,
                                            c0:c0 + ncc]
                                if ncopy % 2 == 0:
                                    nc.vector.tensor_scalar_add(
                                        out=dst, in0=src_b, scalar1=bias_ap)
                                else:
                                    nc.scalar.activation(
                                        out=dst, in_=src_b, func=AF.Identity,
                                        bias=bias_ap)
                                ncopy += 1
                    for b in range(B):
                        if c < 2:
                            dstd = out[b, 128 * c:128 * (c + 1)].rearrange(
                                "c h w -> c (h w)")
                            st = ots[b].rearrange("p h w -> p (h w)")
                            nc.sync.dma_start(out=dstd[:, 0:2048],
                                              in_=st[:, 0:2048])
                            nc.scalar.dma_start(out=dstd[:, 2048:4096],
                                                in_=st[:, 2048:4096])
                        else:
                            p0 = 64 * b
                            dstd = out[b, 256:320].rearrange("c h w -> c (h w)")
                            st = shared.rearrange("p h w -> p (h w)")
                            nc.sync.dma_start(out=dstd[:, 0:2048],
                                              in_=st[p0:p0 + 64, 0:2048])
                            nc.scalar.dma_start(out=dstd[:, 2048:4096],
                                                in_=st[p0:p0 + 64, 2048:4096])
    nc.compile()
    return nc


_cache = {}


def kernel(**inputs):
    x = np.ascontiguousarray(np.asarray(inputs["x"], np.float32))
    params = {k: np.asarray(v, np.float32) for k, v in inputs.items()
              if k != "x"}

    key = hash(tuple(sorted((k, v.tobytes()) for k, v in params.items())))
    if key not in _cache:
        _cache[key] = build(params)
    nc = _cache[key]

    in_maps = [{"x": np.ascontiguousarray(x[B * i:B * (i + 1)])}
               for i in range(NCORES)]
    res = run_bass_kernel_spmd(nc, in_maps, core_ids=list(range(NCORES)),
                               trace=KERNEL_TRACE)
    out = np.concatenate([res.results[i]["out"] for i in range(NCORES)], axis=0)
    if KERNEL_TRACE:
        kernel.last_result = res
    return out


# revision 18
# speedup vs baseline: 1.9949x; 1.0948x over previous
"""Trainium2 Bass kernel for the nn_Adaptor problem.

Computation (per batch image):
  avgpool4x4 -> GN(32 groups)+SiLU -> conv3x3 320->8 -> attention(4 heads) ->
  per-pixel LN + MLP -> GN(8)+SiLU -> upsample x4 nearest -> conv3x3 8->320

Distribution: pure data parallel over batch. 16 images / 8 cores = 2 per core.
Params are baked into the NEFF as inline consts (recomputed from the numpy
arrays passed to kernel() at trace time).

Implementation notes:
  - pooling keeps raw 4x4 sums (16x scale); GN1 uses eps_eff = 256*eps so the
    normalized output is exact.
  - GN1 group stats via per-channel bn_stats + grouping-matrix matmuls on PE.
  - All norm+SiLU applications fused into single scalar-engine activations.
  - conv1 as 9 shifted-window matmuls over a zero-padded 18x18 tile; both
    local batch images stacked along the matmul free dim (N=512).
  - attention: transposed scores E^T = exp(k^T q) without max subtraction
    (|scores| < 0.5 for this operator family); two heads per matmul via
    zero-masked q blocks; softmax denominators via ones-matmul column sums;
    head gather folded into zero-masked projection matmuls.
  - GN2 applied pre-upsample (nearest-upsample preserves per-channel stats).
  - conv2-after-upsample collapses to 9 phase groups with collapsed weights on
    the 16x16 grid (K=72 over a 9-slot shifted-window stack); phase outputs are
    interleaved on-chip by strided copies with step-0 column duplication, then
    written out with fully contiguous DMAs split across both HWDGE rings.
  - the middle is one batch-stacked dependency chain; engine streams execute
    in order, so fewer/wider ops beat two interleaved per-batch chains.
"""

import ml_dtypes
import numpy as np

import concourse.bass as bass
import concourse.bacc as bacc
import concourse.tile as tile
from concourse import mybir
from concourse.bass_utils import run_bass_kernel_spmd

F32 = mybir.dt.float32
BF16 = mybir.dt.bfloat16
NPBF = ml_dtypes.bfloat16
AF = mybir.ActivationFunctionType
ALU = mybir.AluOpType
AX = mybir.AxisListType

CH, C, D, HEADS = 320, 4, 8, 4
EPS = 1e-5
B = 2
N = 256
NCORES = 8

TAPSETS = {
    0: [(0, (0,)), (1, (1, 2))],
    1: [(1, (0, 1, 2))],
    3: [(1, (0, 1)), (2, (2,))],
}
GROUPS = [(pi, pj) for pi in (0, 1, 3) for pj in (0, 1, 3)]
ROWSETS = {0: (0, 1), 1: (1, 2), 3: (3, 1)}   # (start row, duplication count)

KERNEL_TRACE = False


def _conv2_tables(w_conv2):
    """W9 [72, 9, 3, 128]: collapsed per-phase-group weights over the 9-slot
    shifted-window stack; chunk 2 duplicated into cols 64:128 (two-batch
    chunk-2 matmul keeps batch 1 at psum partitions 64:128)."""
    W9 = np.zeros((72, 9, 3, 128), np.float32)
    for g, (pi, pj) in enumerate(GROUPS):
        for (dy, kys) in TAPSETS[pi]:
            for (dx, kxs) in TAPSETS[pj]:
                s = 3 * dy + dx
                wsum = np.zeros((CH, D), np.float32)
                for ky in kys:
                    for kx in kxs:
                        wsum += w_conv2[:, :, ky, kx]
                for c in range(3):
                    oc0 = 128 * c
                    ocn = min(128, CH - oc0)
                    blk = wsum[oc0:oc0 + ocn].T
                    W9[8 * s:8 * s + 8, g, c, :ocn] += blk
                    if c == 2:
                        W9[8 * s:8 * s + 8, g, c, 64:64 + ocn] += blk
    return W9


def build(params):
    P = params
    nc = bacc.Bacc("TRN2")

    x = nc.dram_tensor("x", [B, CH, 64, 64], F32, kind="ExternalInput")
    out = nc.dram_tensor("out", [B, CH, 64, 64], F32, kind="ExternalOutput")

    # ---------------- host-side constant prep ----------------
    s = float(1 / np.sqrt(D // HEADS))
    wq = P["w_qkv"].copy()
    bq = P["b_qkv"].copy()
    wq[D:2 * D] *= s
    bq[D:2 * D] *= s

    W1 = np.zeros((128, 3, 9, 8), np.float32)
    for c in range(3):
        c0 = 128 * c
        cn = min(128, CH - c0)
        for ky in range(3):
            for kx in range(3):
                W1[:cn, c, 3 * ky + kx, :] = P["w_conv1"][:, c0:c0 + cn, ky, kx].T

    W9 = _conv2_tables(P["w_conv2"])

    Gsum = np.zeros((128, 3, 32), np.float32)
    GT = np.zeros((32, 3, 128), np.float32)
    for c in range(CH):
        k, p = divmod(c, 128)
        Gsum[p, k, c // 10] = 0.1
        GT[c // 10, k, p] = 1.0

    def chunks(v, dup2=False):
        a = np.zeros((128, 3), np.float32)
        for c in range(3):
            c0 = 128 * c
            cn = min(128, CH - c0)
            a[:cn, c] = v[c0:c0 + cn]
            if dup2 and c == 2:
                a[64:64 + cn, c] = v[c0:c0 + cn]
        return a

    vch = np.stack([chunks(P["g1"]), chunks(P["b1"]),
                    chunks(P["b_conv2"], dup2=True)], axis=2)  # [128, 3, 3]

    cols8, pk8 = {}, []

    def pack8(name, arr):
        arr = np.asarray(arr, np.float32).reshape(8, -1)
        cols8[name] = (sum(a.shape[1] for a in pk8), arr.shape[1])
        pk8.append(arr)

    pack8("ones8", np.full((8, 1), 0.125, np.float32))
    for nm, val in [("bq8", bq[0:D]), ("bk8", bq[D:2 * D]), ("bv", bq[2 * D:]),
                    ("b_conv1", P["b_conv1"]), ("b_proj", P["b_proj"]),
                    ("ln_g", P["ln_g"]), ("ln_b", P["ln_b"]),
                    ("b_fc2", P["b_fc2"]), ("ga", P["ga"]), ("ba", P["ba"]),
                    ("g2", P["g2"]), ("b2", P["b2"])]:
        pack8(nm, val.reshape(8, 1))
    PK8 = np.concatenate(pk8, axis=1)

    cols8b, pk8b = {}, []

    def pack8b(name, arr):
        arr = np.asarray(arr, np.float32).reshape(8, -1)
        cols8b[name] = (sum(a.shape[1] for a in pk8b), arr.shape[1])
        pk8b.append(arr)

    pack8b("wqT", wq[0:D].T)
    pack8b("wkT", wq[D:2 * D].T)
    pack8b("wvT", P["w_qkv"][2 * D:3 * D].T)
    pack8b("ident8", np.eye(8, dtype=np.float32))
    qmask = np.zeros((8, 4), np.float32)
    for c in range(8):
        qmask[c, c // 2] = 1.0
    pack8b("qmask", qmask)
    wpH = np.zeros((8, 4, 8), np.float32)
    for c in range(8):
        wpH[c, c // 2, :] = P["w_proj"][:, c]
    pack8b("wpH", wpH.reshape(8, 32))
    pack8b("wf1T", P["w_fc1"].T)
    PK8B = np.concatenate(pk8b, axis=1).astype(NPBF)

    PK16 = np.concatenate([P["w_fc2"].T, P["b_fc1"].reshape(16, 1)], axis=1)
    WF2B = P["w_fc2"].T.astype(NPBF)

    h_w1 = nc.inline_tensor(W1.astype(NPBF), "cW1")
    h_w9 = nc.inline_tensor(W9.astype(NPBF), "cW9")
    h_gsum = nc.inline_tensor(Gsum, "cGsum")
    h_gt = nc.inline_tensor(GT, "cGT")
    h_vch = nc.inline_tensor(vch, "cVch")
    h_pk8 = nc.inline_tensor(PK8, "cPK8")
    h_pk8b = nc.inline_tensor(PK8B, "cPK8B")
    h_pk16 = nc.inline_tensor(PK16, "cPK16")
    h_wf2b = nc.inline_tensor(WF2B, "cWF2B")
    h_one18 = nc.inline_tensor(np.ones((1, 8), np.float32), "cOne18")
    h_ones128 = nc.inline_tensor(
        np.ones((128, 8), np.float32).astype(NPBF), "cOnes128")

    with tile.TileContext(nc) as tc:
        with (
            tc.tile_pool(name="consts", bufs=1) as csts,
            tc.tile_pool(name="xin", bufs=3) as xin,
            tc.tile_pool(name="pooltmp", bufs=2) as ptmp,
            tc.tile_pool(name="mid", bufs=1) as mid,
            tc.tile_pool(name="et", bufs=4) as etp,
            tc.tile_pool(name="outp", bufs=4) as outp,
            tc.tile_pool(name="psA", bufs=2, space="PSUM") as psA,
            tc.tile_pool(name="psT", bufs=2, space="PSUM") as psT,
            tc.tile_pool(name="psAcc", bufs=2, space="PSUM") as psAcc,
            tc.tile_pool(name="psC", bufs=2, space="PSUM") as psC,
        ):
            # ---------------- consts ----------------
            w1t = csts.tile([128, 3, 9, 8], BF16)
            nc.gpsimd.dma_start(out=w1t, in_=h_w1[:])
            w9t = csts.tile([72, 9, 3, 128], BF16)
            nc.gpsimd.dma_start(out=w9t, in_=h_w9[:])
            gsumt = csts.tile([128, 3, 32], F32)
            nc.gpsimd.dma_start(out=gsumt, in_=h_gsum[:])
            gtt = csts.tile([32, 3, 128], F32)
            nc.gpsimd.dma_start(out=gtt, in_=h_gt[:])
            vcht = csts.tile([128, 3, 3], F32)
            nc.gpsimd.dma_start(out=vcht, in_=h_vch[:])
            pk8t = csts.tile([8, PK8.shape[1]], F32)
            nc.gpsimd.dma_start(out=pk8t, in_=h_pk8[:])
            pk8bt = csts.tile([8, PK8B.shape[1]], BF16)
            nc.gpsimd.dma_start(out=pk8bt, in_=h_pk8b[:])
            pk16t = csts.tile([16, 9], F32)
            nc.gpsimd.dma_start(out=pk16t, in_=h_pk16[:])
            wf2bt = csts.tile([16, 8], BF16)
            nc.gpsimd.dma_start(out=wf2bt, in_=h_wf2b[:])
            one18t = csts.tile([1, 8], F32)
            nc.gpsimd.dma_start(out=one18t, in_=h_one18[:])
            ones8x = csts.tile([128, 8], BF16)
            nc.gpsimd.dma_start(out=ones8x, in_=h_ones128[:])

            def c8(name):
                c0, w = cols8[name]
                return pk8t[:, c0:c0 + w]

            def c8b(name):
                c0, w = cols8b[name]
                return pk8bt[:, c0:c0 + w]

            bf1 = pk16t[:, 8:9]

            eps1 = csts.tile([32, 1], F32)
            nc.vector.memset(eps1, 256.0 * EPS)
            eps8 = csts.tile([8, 1], F32)
            nc.vector.memset(eps8, EPS)
            eps1p = csts.tile([1, 1], F32)
            nc.vector.memset(eps1p, EPS)

            # ---------------- state ----------------
            z0 = mid.tile([128, 3, B, N], F32)
            nc.vector.memset(z0[64:128, 2, :, :], 0.0)
            stat2 = mid.tile([128, 3, B, 2], F32)
            ab = mid.tile([32, 2 * B], F32)
            sbias = mid.tile([128, 3, B, 2], F32)
            zc = mid.tile([8, B, N], F32)
            zna = mid.tile([8, B, N], BF16)
            qsb = mid.tile([8, B, N], BF16)
            ksb = mid.tile([8, B, N], BF16)
            vsb = mid.tile([8, B, N], BF16)
            qblk = mid.tile([8, B, 2, 2, N], BF16)
            vT = mid.tile([128, 2, B, 8], BF16)
            z1 = mid.tile([8, B, N], F32)
            z2 = mid.tile([8, B, N], F32)
            pad1 = mid.tile([128, 3, B, 18, 18], BF16)
            spad = mid.tile([8, B, 18, 18], BF16)
            sp9 = mid.tile([72, B, N], BF16)

            # ---------------- phase 1: load + pool ----------------
            def pool(xt, dst):
                wp = ptmp.tile([128, 1024], F32, tag="wp")
                nc.vector.reduce_sum(
                    out=wp, in_=xt.rearrange("p (a b) -> p a b", b=4), axis=AX.X)
                wpv = wp.rearrange("p (hb hi wb) -> p hb hi wb", hi=4, wb=16)
                t01 = ptmp.tile([128, 16, 16], F32, tag="t01")
                nc.gpsimd.tensor_add(t01, wpv[:, :, 0, :], wpv[:, :, 1, :])
                t23 = ptmp.tile([128, 16, 16], F32, tag="t23")
                nc.gpsimd.tensor_add(t23, wpv[:, :, 2, :], wpv[:, :, 3, :])
                nc.gpsimd.tensor_add(dst, t01, t23)

            loads = [(0, 0), (0, 1), (None, 2), (1, 0), (1, 1)]
            z0c2 = ptmp.tile([128, N], F32, tag="z0c2")
            with nc.named_scope("pool"):
                for b, k in loads:
                    xt = xin.tile([128, 4096], F32, tag="xt")
                    if b is not None:
                        src_ap = x[b, 128 * k:128 * (k + 1)].rearrange(
                            "c h w -> c (h w)")
                        nc.sync.dma_start(out=xt[:, 0:2048], in_=src_ap[:, 0:2048])
                        nc.scalar.dma_start(out=xt[:, 2048:4096],
                                            in_=src_ap[:, 2048:4096])
                        pool(xt, z0[:, k, b, :])
                    else:
                        for bb in range(2):
                            src_ap = x[bb, 256:320].rearrange("c h w -> c (h w)")
                            eng = nc.sync if bb == 0 else nc.scalar
                            eng.dma_start(out=xt[64 * bb:64 * bb + 64, :],
                                          in_=src_ap)
                        pool(xt, z0c2)
                        nc.gpsimd.dma_start(out=z0[0:64, 2, 0, :],
                                            in_=z0c2[0:64, :])
                        nc.gpsimd.dma_start(out=z0[0:64, 2, 1, :],
                                            in_=z0c2[64:128, :])

            # ---------------- middle: one batch-stacked chain ----------------
            with nc.named_scope("middle"):
                # GN1 per-channel stats
                for k in range(3):
                    for b in range(B):
                        st6 = ptmp.tile([128, 6], F32, tag="st6")
                        nc.vector.bn_stats(out=st6, in_=z0[:, k, b, :])
                        nc.vector.bn_aggr(out=stat2[:, k, b, :], in_=st6)
                    tm = ptmp.tile([128, 2], F32, tag="tm")
                    nc.vector.tensor_mul(tm, stat2[:, k, :, 0], stat2[:, k, :, 0])
                    nc.vector.tensor_add(stat2[:, k, :, 1], stat2[:, k, :, 1], tm)
                pg = psA.tile([32, 4], F32, tag="ps")
                for k in range(3):
                    nc.tensor.matmul(
                        pg, gsumt[:, k, :],
                        stat2[:, k, :, :].rearrange("p a b -> p (a b)"),
                        start=(k == 0), stop=(k == 2))
                gm = ptmp.tile([32, 4], F32, tag="gm")
                nc.vector.tensor_copy(gm, pg)
                gv = ptmp.tile([32, 2], F32, tag="gv")
                nc.vector.tensor_mul(gv, gm[:, 0::2], gm[:, 0::2])
                nc.vector.tensor_sub(gv, gm[:, 1::2], gv)
                nc.scalar.activation(out=gv, in_=gv, func=AF.Sqrt, bias=eps1)
                nc.vector.reciprocal(out=ab[:, 0::2], in_=gv)
                nc.vector.tensor_copy(ab[:, 1::2], gm[:, 0::2])
                for k in range(3):
                    pbc = psA.tile([128, 4], F32, tag="ps")
                    nc.tensor.matmul(pbc, gtt[:, k, :], ab,
                                     start=True, stop=True)
                    nc.vector.tensor_scalar_mul(
                        out=sbias[:, k, :, 0], in0=pbc[:, 0::2],
                        scalar1=vcht[:, k, 0:1])
                    tm2 = ptmp.tile([128, 2], F32, tag="tm2")
                    nc.vector.tensor_mul(tm2, sbias[:, k, :, 0], pbc[:, 1::2])
                    nc.vector.tensor_scalar(
                        out=sbias[:, k, :, 1], in0=tm2,
                        scalar1=vcht[:, k, 1:2], scalar2=-1.0,
                        op0=ALU.subtract, op1=ALU.mult)
                nc.gpsimd.memset(pad1, 0.0)
                for k in range(3):
                    for b in range(B):
                        nc.scalar.activation(
                            out=pad1[:, k, b, 1:17, 1:17],
                            in_=z0[:, k, b, :].rearrange("p (h w) -> p h w", w=16),
                            func=AF.Silu,
                            scale=sbias[:, k, b, 0:1], bias=sbias[:, k, b, 1:2])
                # conv1 (27 accumulating matmuls, both batches in N)
                pz = psA.tile([8, B, N], F32, tag="ps")
                first = True
                for k in range(3):
                    for ky in range(3):
                        for kx in range(3):
                            nc.tensor.matmul(
                                pz.rearrange("p a b -> p (a b)"),
                                w1t[:, k, 3 * ky + kx, :],
                                pad1[:, k, :, ky:ky + 16, kx:kx + 16],
                                start=first,
                                stop=(k == 2 and ky == 2 and kx == 2))
                            first = False
                nc.vector.tensor_scalar_add(out=zc, in0=pz,
                                            scalar1=c8("b_conv1"))

                # ---- attention ----
                st6a = ptmp.tile([8, B, 6], F32, tag="st6a")
                mva = ptmp.tile([8, B, 2], F32, tag="mva")
                for b in range(B):
                    nc.vector.bn_stats(out=st6a[:, b, :], in_=zc[:, b, :])
                    nc.vector.bn_aggr(out=mva[:, b, :], in_=st6a[:, b, :])
                ra = ptmp.tile([8, B], F32, tag="ra")
                nc.scalar.activation(out=ra, in_=mva[:, :, 1], func=AF.Sqrt,
                                     bias=eps8)
                nc.vector.reciprocal(out=ra, in_=ra)
                sca = ptmp.tile([8, B, 2], F32, tag="sca")
                nc.vector.tensor_scalar_mul(out=sca[:, :, 0], in0=ra,
                                            scalar1=c8("ga"))
                tm3 = ptmp.tile([8, B], F32, tag="tm3")
                nc.vector.tensor_mul(tm3, sca[:, :, 0], mva[:, :, 0])
                nc.vector.tensor_scalar(
                    out=sca[:, :, 1], in0=tm3, scalar1=c8("ba"), scalar2=-1.0,
                    op0=ALU.subtract, op1=ALU.mult)
                for b in range(B):
                    nc.scalar.activation(out=zna[:, b, :], in_=zc[:, b, :],
                                         func=AF.Identity,
                                         scale=sca[:, b, 0:1],
                                         bias=sca[:, b, 1:2])
                znaf = zna.rearrange("p a b -> p (a b)")
                for wname, bname, dst in [("wqT", "bq8", qsb),
                                          ("wkT", "bk8", ksb),
                                          ("wvT", "bv", vsb)]:
                    pqkv = psA.tile([8, B, N], F32, tag="ps",
                                    name=f"pqkv_{wname}")
                    nc.tensor.matmul(pqkv.rearrange("p a b -> p (a b)"),
                                     c8b(wname), znaf, start=True, stop=True)
                    nc.vector.tensor_scalar_add(out=dst, in0=pqkv,
                                                scalar1=c8(bname))
                qmt = c8b("qmask").rearrange("p (a c) -> p a c", a=2)
                for b in range(B):
                    nc.vector.tensor_mul(
                        qblk[:, b],
                        qsb[:, b, :].unsqueeze(1).unsqueeze(1).broadcast_to(
                            [8, 2, 2, N]),
                        qmt.unsqueeze(3).broadcast_to([8, 2, 2, N]))
                for b in range(B):
                    for mc in range(2):
                        pvt = psT.tile([128, 2, N], F32, tag="pt")
                        nc.tensor.matmul(pvt[:, 0, 0:8],
                                         vsb[:, b, 128 * mc:128 * (mc + 1)],
                                         c8b("ident8"), start=True, stop=True)
                        nc.vector.tensor_copy(vT[:, mc, b, :], pvt[:, 0, 0:8])
                wpH_t = c8b("wpH").rearrange("p (a c) -> p a c", a=4)
                pp = psA.tile([8, B, N], F32, tag="ps")
                for b in range(B):
                    for blk in range(2):
                        psum_s = psAcc.tile([8, 2, N], F32, tag="acc",
                                            name=f"psum_s{b}{blk}")
                        psum_e = psAcc.tile([8, 2, N], F32, tag="acc",
                                            name=f"psum_e{b}{blk}")
                        for mc in range(2):
                            pst = psT.tile([128, 2, N], F32, tag="pt")
                            nc.tensor.matmul(
                                pst.rearrange("p a n -> p (a n)"),
                                ksb[:, b, 128 * mc:128 * (mc + 1)],
                                qblk[:, b, blk].rearrange("p a n -> p (a n)"),
                                start=True, stop=True)
                            et = etp.tile([128, 2, N], BF16, tag="et")
                            nc.scalar.activation(out=et, in_=pst, func=AF.Exp)
                            etf = et.rearrange("p a n -> p (a n)")
                            nc.tensor.matmul(
                                psum_s.rearrange("p a n -> p (a n)"),
                                ones8x, etf, start=(mc == 0), stop=(mc == 1))
                            nc.tensor.matmul(
                                psum_e.rearrange("p a n -> p (a n)"),
                                vT[:, mc, b, :], etf,
                                start=(mc == 0), stop=(mc == 1))
                        den = ptmp.tile([8, 2, N], F32, tag="den")
                        nc.vector.reciprocal_approx_fast(out=den, in_=psum_s)
                        aoblk = ptmp.tile([8, 2, N], BF16, tag="aoblk")
                        nc.vector.tensor_mul(aoblk, psum_e, den)
                        # cross-head lanes: finite garbage x zero proj weight
                        for hp in range(2):
                            nc.tensor.matmul(pp[:, b, :],
                                             wpH_t[:, 2 * blk + hp, :],
                                             aoblk[:, hp, :],
                                             start=(blk == 0 and hp == 0),
                                             stop=(blk == 1 and hp == 1))
                nc.vector.tensor_scalar_add(out=z1, in0=pp, scalar1=c8("b_proj"))
                nc.vector.tensor_add(z1, z1, zc)

                # ---- per-pixel LN + MLP (batch-stacked) ----
                z1f = z1.rearrange("p a b -> p (a b)")
                sq8 = ptmp.tile([8, B, N], F32, tag="sq8")
                nc.vector.tensor_mul(sq8, z1, z1)
                pln = psA.tile([33, B, N], F32, tag="ps")
                nc.tensor.matmul(pln[0:1, :, :].rearrange("p a b -> p (a b)"),
                                 c8("ones8"), z1f, start=True, stop=True)
                nc.tensor.matmul(pln[32:33, :, :].rearrange("p a b -> p (a b)"),
                                 c8("ones8"), sq8.rearrange("p a b -> p (a b)"),
                                 start=True, stop=True)
                murs = ptmp.tile([1, 2, B, N], F32, tag="murs")
                nc.vector.tensor_copy(murs[:, 0], pln[0:1, :, :])
                musq = ptmp.tile([1, B, N], F32, tag="musq")
                nc.vector.tensor_mul(musq, murs[:, 0], murs[:, 0])
                ex2 = ptmp.tile([1, B, N], F32, tag="ex2")
                nc.vector.tensor_sub(ex2, pln[32:33, :, :], musq)
                nc.scalar.activation(out=ex2, in_=ex2, func=AF.Sqrt, bias=eps1p)
                nc.vector.reciprocal_approx_fast(out=murs[:, 1], in_=ex2)
                pbr_mu = psA.tile([8, B, N], F32, tag="ps", name="pbr_mu")
                nc.tensor.matmul(pbr_mu.rearrange("p a b -> p (a b)"), one18t,
                                 murs[:, 0].rearrange("p a b -> p (a b)"),
                                 start=True, stop=True)
                pbr_rs = psA.tile([8, B, N], F32, tag="ps", name="pbr_rs")
                nc.tensor.matmul(pbr_rs.rearrange("p a b -> p (a b)"), one18t,
                                 murs[:, 1].rearrange("p a b -> p (a b)"),
                                 start=True, stop=True)
                cen = ptmp.tile([8, B, N], F32, tag="cen")
                nc.vector.tensor_sub(cen, z1, pbr_mu)
                nc.vector.tensor_mul(cen, cen, pbr_rs)
                lnt = ptmp.tile([8, B, N], BF16, tag="lnt")
                nc.vector.tensor_scalar(
                    out=lnt, in0=cen, scalar1=c8("ln_g"), scalar2=c8("ln_b"),
                    op0=ALU.mult, op1=ALU.add)
                pf1 = psA.tile([16, B, N], F32, tag="ps")
                nc.tensor.matmul(pf1.rearrange("p a b -> p (a b)"), c8b("wf1T"),
                                 lnt.rearrange("p a b -> p (a b)"),
                                 start=True, stop=True)
                hmid = ptmp.tile([16, B, N], BF16, tag="hmid")
                nc.scalar.activation(out=hmid, in_=pf1, func=AF.Gelu, bias=bf1)
                pf2 = psA.tile([8, B, N], F32, tag="ps")
                nc.tensor.matmul(pf2.rearrange("p a b -> p (a b)"), wf2bt,
                                 hmid.rearrange("p a b -> p (a b)"),
                                 start=True, stop=True)
                nc.vector.tensor_scalar_add(out=z2, in0=pf2, scalar1=c8("b_fc2"))
                nc.vector.tensor_add(z2, z2, z1)

                # ---- GN2 + SiLU into padded tile ----
                st6b = ptmp.tile([8, B, 6], F32, tag="st6b")
                mvb = ptmp.tile([8, B, 2], F32, tag="mvb")
                for b in range(B):
                    nc.vector.bn_stats(out=st6b[:, b, :], in_=z2[:, b, :])
                    nc.vector.bn_aggr(out=mvb[:, b, :], in_=st6b[:, b, :])
                rb2 = ptmp.tile([8, B], F32, tag="rb2")
                nc.scalar.activation(out=rb2, in_=mvb[:, :, 1], func=AF.Sqrt,
                                     bias=eps8)
                nc.vector.reciprocal(out=rb2, in_=rb2)
                scb = ptmp.tile([8, B, 2], F32, tag="scb")
                nc.vector.tensor_scalar_mul(out=scb[:, :, 0], in0=rb2,
                                            scalar1=c8("g2"))
                tm4 = ptmp.tile([8, B], F32, tag="tm4")
                nc.vector.tensor_mul(tm4, scb[:, :, 0], mvb[:, :, 0])
                nc.vector.tensor_scalar(
                    out=scb[:, :, 1], in0=tm4, scalar1=c8("b2"), scalar2=-1.0,
                    op0=ALU.subtract, op1=ALU.mult)
                nc.gpsimd.memset(spad, 0.0)
                for b in range(B):
                    nc.scalar.activation(
                        out=spad[:, b, 1:17, 1:17],
                        in_=z2[:, b, :].rearrange("p (h w) -> p h w", w=16),
                        func=AF.Silu, scale=scb[:, b, 0:1], bias=scb[:, b, 1:2])
                for dy in range(3):
                    for dx in range(3):
                        slot = 3 * dy + dx
                        for b in range(B):
                            eng = nc.sync if (slot + b) % 2 == 0 else nc.scalar
                            eng.dma_start(
                                out=sp9[8 * slot:8 * slot + 8, b, :],
                                in_=spad[:, b, dy:dy + 16, dx:dx + 16])

            # ---------------- conv2 + interleave + out ----------------
            sp9f = sp9.rearrange("p a b -> p (a b)")
            with nc.named_scope("conv2"):
                for c in range(3):
                    if c < 2:
                        ots = [outp.tile([128, 64, 64], F32, tag="oc",
                                         name=f"oc_b{bb}c{c}")
                               for bb in range(B)]
                    else:
                        shared = outp.tile([128, 64, 64], F32, tag="oc",
                                           name="oc_c2")
                        ots = [shared, shared]
                    for g, (pi, pj) in enumerate(GROUPS):
                        r0, nr = ROWSETS[pi]
                        c0, ncc = ROWSETS[pj]
                        pcv = psC.tile([128, B, 16, 16], F32, tag="pcv")
                        nc.tensor.matmul(
                            pcv.rearrange("p a b c -> p (a b c)"),
                            w9t[:, g, c, :], sp9f, start=True, stop=True)
                        ncopy = 0
                        for b in range(B):
                            if c < 2:
                                p0, pn = 0, 128
                            else:
                                p0, pn = 64 * b, 64
                            src_b = pcv[p0:p0 + pn, b].unsqueeze(3).broadcast_to(
                                [pn, 16, 16, ncc])
                            base5 = ots[b].rearrange(
                                "p (bi ri) (bj rj) -> p bi ri bj rj",
                                ri=4, rj=4)
                            bias_ap = vcht[p0:p0 + pn, c, 2:3]
                            for rr in range(nr):
                                dst = base5[p0:p0 + pn, :, r0 + rr, :,
                                            c0:c0 + ncc]
                                if ncopy % 2 == 0:
                                    nc.vector.tensor_scalar_add(
                                        out=dst, in0=src_b, scalar1=bias_ap)
                                else:
                                    nc.scalar.activation(
                                        out=dst, in_=src_b, func=AF.Identity,
                                        bias=bias_ap)
                                ncopy += 1
                    for b in range(B):
                        if c < 2:
                            dstd = out[b, 128 * c:128 * (c + 1)].rearrange(
                                "c h w -> c (h w)")
                            st = ots[b].rearrange("p h w -> p (h w)")
                            nc.sync.dma_start(out=dstd[:, 0:2048],
                                              in_=st[:, 0:2048])
                            nc.scalar.dma_start(out=dstd[:, 2048:4096],
                                                in_=st[:, 2048:4096])
                        else:
                            p0 = 64 * b
                            dstd = out[b, 256:320].rearrange("c h w -> c (h w)")
                            st = shared.rearrange("p h w -> p (h w)")
                            nc.sync.dma_start(out=dstd[:, 0:2048],
                                              in_=st[p0:p0 + 64, 0:2048])
                            nc.scalar.dma_start(out=dstd[:, 2048:4096],
                                                in_=st[p0:p0 + 64, 2048:4096])
    nc.compile()
    return nc


_cache = {}


def kernel(**inputs):
    x = np.ascontiguousarray(np.asarray(inputs["x"], np.float32))
    params = {k: np.asarray(v, np.float32) for k, v in inputs.items()
              if k != "x"}

    key = hash(tuple(sorted((k, v.tobytes()) for k, v in params.items())))
    if key not in _cache:
        _cache[key] = build(params)
    nc = _cache[key]

    in_maps = [{"x": np.ascontiguousarray(x[B * i:B * (i + 1)])}
               for i in range(NCORES)]
    res = run_bass_kernel_spmd(nc, in_maps, core_ids=list(range(NCORES)),
                               trace=KERNEL_TRACE)
    out = np.concatenate([res.results[i]["out"] for i in range(NCORES)], axis=0)
    if KERNEL_TRACE:
        kernel.last_result = res
    return out
